# revision 11
# baseline (speedup 1.0000x reference)
"""LoraLinear (int8-dequant matmul + low-rank LoRA) on 8 trn2 NeuronCores.

out[b,s,o] = sum_i x[b,s,i]*q[o,i]*scale[o] + 2.0 * sum_r (sum_i x[b,s,i]*A[r,i]) * B[o,r]

Strategy: data-parallel over the 8192 flattened tokens (1024/core, no
collectives). The LoRA update is dense-folded on the host into the
effective weight W_eff = q*scale + 2*B@A, so the device does a single
GEMM. W_eff and x are each split into two exact-ish fp8 e4m3 planes
(hi = rne(v), lo = rne(v - hi)); three of the four plane cross-products
are computed (hi*hi + lo*hi + hi*lo), leaving only the lo*lo term as
error (~1.3e-3 rel). All matmuls run in DoubleRow perf mode (fp8,
K=256 per instruction, 0.5 cycles per output element = 4x bf16 MAC
throughput), accumulating the three passes in fp32 PSUM before one
eviction per output tile.

Pipeline details: 8 persistent PSUM tiles (one per token tile) give
precise per-bank WAR deps across output-column tiles; a dozen dummy
matmuls on a zeroed SBUF tile warm the PE p-state ramp during the
prologue DMA; ot=0 runs all three passes kg-streamed behind the loads
with a tt-outer tail so evictions spread; later ots prefetch weights
one tile ahead.
"""

import numpy as np
import ml_dtypes

E4 = ml_dtypes.float8_e4m3

B, S, DIN, DOUT, R = 4, 2048, 4096, 4096, 64
N_CORES = 8
TOK = B * S  # 8192
T = TOK // N_CORES  # 1024 tokens per core
P = 128
KG = DIN // 256  # 16 K-groups, each 2x128 contraction per DoubleRow matmul
O_TILE = 512
N_OT = DOUT // O_TILE  # 8
N_TT = T // P  # 8
WCH = 2  # kg per W DMA chunk
NCH = KG // WCH  # 8 W chunks per (plane, ot)
SCALING = 2.0
N_WARM = 30  # PE p-state warmup matmuls (128-wide, end ~ when data lands)
KG_TAIL = 2  # kg processed tt-outer at the end of ot=0

_CACHE = {}


def build_nc():
    import concourse.mybir as mybir
    import concourse.tile as tile
    from concourse import bacc

    dt = mybir.dt
    DR = mybir.MatmulPerfMode.DoubleRow
    nc = bacc.Bacc("TRN2", target_bir_lowering=False, debug=False,
                   num_devices=N_CORES)

    xq_d = nc.dram_tensor("xq", [P, KG, 2, T], dt.float8e4, kind="ExternalInput").ap()
    xr_d = nc.dram_tensor("xr", [P, KG, 2, T], dt.float8e4, kind="ExternalInput").ap()
    wq_d = nc.dram_tensor("wq", [N_OT, P, KG, 2, O_TILE], dt.float8e4, kind="ExternalInput").ap()
    wr_d = nc.dram_tensor("wr", [N_OT, P, KG, 2, O_TILE], dt.float8e4, kind="ExternalInput").ap()
    out_d = nc.dram_tensor("out", [N_OT, N_TT, P, O_TILE], dt.float32, kind="ExternalOutput").ap()

    with tile.TileContext(nc) as tc:
        with (
            tc.tile_pool(name="xpool", bufs=1) as xpool,
            tc.tile_pool(name="wpool", bufs=2) as wpool,
            tc.tile_pool(name="opool", bufs=4) as opool,
            tc.tile_pool(name="pspool", bufs=1, space="PSUM") as pspool,
        ):
            # persistent PSUM tiles, one per token tile; reused every ot so
            # WAR deps are per-bank (matmul waits only on its own bank's
            # eviction, not a pool-rotation barrier)
            ps = [pspool.tile([P, O_TILE], dt.float32, tag=f"ps{t}", name=f"ps{t}")
                  for t in range(N_TT)]

            # warmup: PE ramps to full p-state during the prologue DMAs
            z = xpool.tile([P, 2, P], dt.float8e4, tag="z", name="z")
            nc.vector.memset(z[:], 0)
            for i in range(N_WARM):
                nc.tensor.matmul(ps[0][:, :P], z[:], z[:],
                                 start=True, stop=True, perf_mode=DR)

            xq_t = [xpool.tile([P, 2, T], dt.float8e4, tag=f"xq{k}", name=f"xq{k}")
                    for k in range(KG)]
            xr_t = [xpool.tile([P, 2, T], dt.float8e4, tag=f"xr{k}", name=f"xr{k}")
                    for k in range(KG)]

            def alloc_w(ot):
                wq = [wpool.tile([P, WCH, 2, O_TILE], dt.float8e4, tag=f"wq{c}",
                                 name=f"wq{ot}_{c}") for c in range(NCH)]
                wr = [wpool.tile([P, WCH, 2, O_TILE], dt.float8e4, tag=f"wr{c}",
                                 name=f"wr{ot}_{c}") for c in range(NCH)]
                return wq, wr

            def dma_w_chunk(ws, w_d, ot, c):
                nc.sync.dma_start(ws[c][:], w_d[ot, :, WCH * c:WCH * (c + 1), :, :])

            def w_sl(ws, kg):
                return ws[kg // WCH][:, kg % WCH, :, :]

            # prologue DMA: interleaved in exactly the order ot=0 consumes
            w0q, w0r = alloc_w(0)
            for c in range(NCH):
                dma_w_chunk(w0q, wq_d, 0, c)
                nc.sync.dma_start(xq_t[2 * c][:], xq_d[:, 2 * c, :, :])
                dma_w_chunk(w0r, wr_d, 0, c)
                nc.sync.dma_start(xr_t[2 * c][:], xr_d[:, 2 * c, :, :])
                nc.sync.dma_start(xq_t[2 * c + 1][:], xq_d[:, 2 * c + 1, :, :])
                nc.sync.dma_start(xr_t[2 * c + 1][:], xr_d[:, 2 * c + 1, :, :])

            # W[1] prefetch issues right behind the prologue (ahead of ot0's
            # eviction stores in the in-order DMA queue)
            w1q, w1r = alloc_w(1)
            for c in range(NCH):
                dma_w_chunk(w1q, wq_d, 1, c)
                dma_w_chunk(w1r, wr_d, 1, c)

            def evict(tt, ot, last=False):
                # separate staging tiles so the DVE and ACT copies run in
                # parallel (same tile would serialize on tile-granular WAW)
                h = O_TILE // 2
                sa = opool.tile([P, h], dt.float32, tag="sta", name=f"sta{ot}_{tt}")
                sb = opool.tile([P, O_TILE - h], dt.float32, tag="stb", name=f"stb{ot}_{tt}")
                if last:
                    # emit the small ACT half first so its store wins the
                    # in-order DMA issue queue and the tail chain is short
                    nc.scalar.copy(sb[:], ps[tt][:, h:])
                    nc.sync.dma_start(out_d[ot, tt, :, h:O_TILE], sb[:])
                    nc.vector.tensor_copy(out=sa[:], in_=ps[tt][:, :h])
                    nc.sync.dma_start(out_d[ot, tt, :, 0:h], sa[:])
                else:
                    nc.vector.tensor_copy(out=sa[:], in_=ps[tt][:, :h])
                    nc.sync.dma_start(out_d[ot, tt, :, 0:h], sa[:])
                    nc.scalar.copy(sb[:], ps[tt][:, h:])
                    nc.sync.dma_start(out_d[ot, tt, :, h:O_TILE], sb[:])

            PASSES = ((xq_t, "q"), (xq_t, "r"), (xr_t, "q"))

            def mm(tt, kg, xp, wsel, wq, wr, start, stop):
                nc.tensor.matmul(
                    ps[tt][:], xp[kg][:, :, tt * P:(tt + 1) * P],
                    w_sl(wq if wsel == "q" else wr, kg),
                    start=start, stop=stop, perf_mode=DR,
                )

            # ---- ot = 0: kg-streamed, all 3 passes per kg; last KG_TAIL
            # kgs go tt-outer so the 8 evictions spread out
            for k in range(KG - KG_TAIL):
                for pi, (xp, wsel) in enumerate(PASSES):
                    for tt in range(N_TT):
                        mm(tt, k, xp, wsel, w0q, w0r,
                           start=(pi == 0 and k == 0), stop=False)
            for tt in range(N_TT):
                for k in range(KG - KG_TAIL, KG):
                    for pi, (xp, wsel) in enumerate(PASSES):
                        mm(tt, k, xp, wsel, w0q, w0r, start=False,
                           stop=(pi == len(PASSES) - 1 and k == KG - 1))
                evict(tt, 0)

            # ---- ot = 1..7: weights prefetched an ot ahead; (q,q)+(r,q)
            # kg-outer, final (q,r) pass tt-outer with spread evictions
            wq_c, wr_c = w1q, w1r
            for ot in range(1, N_OT):
                wq, wr = wq_c, wr_c
                if ot + 1 < N_OT:
                    wq_c, wr_c = alloc_w(ot + 1)
                    for c in range(NCH):
                        dma_w_chunk(wq_c, wq_d, ot + 1, c)
                        dma_w_chunk(wr_c, wr_d, ot + 1, c)
                for k in range(KG):
                    for xp, wsel, first in ((xq_t, "q", True), (xq_t, "r", False)):
                        for tt in range(N_TT):
                            mm(tt, k, xp, wsel, wq, wr,
                               start=(first and k == 0), stop=False)
                for tt in range(N_TT):
                    for k in range(KG):
                        mm(tt, k, xr_t, "q", wq, wr, start=False,
                           stop=(k == KG - 1))
                    evict(tt, ot, last=(ot == N_OT - 1 and tt == N_TT - 1))

    nc.compile()
    return nc


def _split_planes(v):
    hi = v.astype(E4)
    lo = (v - hi.astype(np.float32)).astype(E4)
    return hi, lo


def _prep_inputs(x, qweight, scale, lora_A, lora_B):
    # effective dense weight with the LoRA update folded in
    w = qweight.astype(np.float32) * scale.astype(np.float32)
    w += SCALING * (lora_B.astype(np.float32) @ lora_A.astype(np.float32))
    wq, wr = _split_planes(w)

    def w_layout(p):
        # [DOUT, DIN] -> K-major rhs layout [N_OT, P, KG, 2, O_TILE],
        # K = kg*256 + sub*128 + p
        t = p.T.reshape(KG, 2, P, N_OT, O_TILE)
        return np.ascontiguousarray(t.transpose(3, 2, 0, 1, 4))

    xf = np.ascontiguousarray(x.reshape(TOK, DIN))
    xhi, xlo = _split_planes(xf)

    def x_layout(p, c):
        # core slice [T, DIN] -> lhsT layout [P, KG, 2, T]
        t = p[c * T:(c + 1) * T].T.reshape(KG, 2, P, T)
        return np.ascontiguousarray(t.transpose(2, 0, 1, 3))

    wq_l, wr_l = w_layout(wq), w_layout(wr)
    per_core = [
        {"xq": x_layout(xhi, c), "xr": x_layout(xlo, c), "wq": wq_l, "wr": wr_l}
        for c in range(N_CORES)
    ]
    return per_core


def run(x, qweight, scale, lora_A, lora_B, trace=False):
    from concourse.bass_utils import run_bass_kernel_spmd

    if "nc" not in _CACHE:
        _CACHE["nc"] = build_nc()
    nc = _CACHE["nc"]

    in_maps = _prep_inputs(x, qweight, scale, lora_A, lora_B)
    res = run_bass_kernel_spmd(nc, in_maps, core_ids=list(range(N_CORES)),
                               trace=trace)
    outs = []
    for c in range(N_CORES):
        o = res.results[c]["out"]  # [N_OT, N_TT, P, O_TILE]
        outs.append(o.transpose(1, 2, 0, 3).reshape(T, DOUT))
    full = np.concatenate(outs, axis=0).reshape(B, S, DOUT).astype(np.float32)
    return full, res


def kernel(x, qweight, scale, lora_A, lora_B):
    full, _ = run(x, qweight, scale, lora_A, lora_B)
    return full
